# revision 3
# baseline (speedup 1.0000x reference)
"""NSA-style block compression (sparse_attention) Trainium2 kernel.

y[b, m, h, :] = sum_{r<32} w[r] * (x[b, 16*m + r, h, :] + pe[r, :]),  M = 1023

Decomposition used on device (per core):
  - Shard: 8 cores = 4 batches x 2 sequence-halves. Each core gets a
    contiguous [8208, 512] slice of x[b] (rows = seq positions, cols = H*D)
    and produces 512 output rows ([512, 512]); halves overlap by one output
    row which the host drops.
  - The strided conv is expressed as 17 banded [128,128] (or [16,128] for the
    window tail) weight matrices accumulating into one PSUM bank per 128
    outputs: out[j, f] = sum_u sum_p W_u[p, j] * xtile_{16q+u}[p, f].
  - The positional-encoding term factors out: + (w @ pe) broadcast over m,
    added with one extra rank-1 matmul per PSUM tile.
"""

import os
import sys

sys.path.insert(0, "/opt/trn_rl_repo")

import numpy as np

_B, _N, _H, _D = 4, 16384, 4, 128
_K, _S = 32, 16
_M = (_N - _K) // _S + 1          # 1023
_F = _H * _D                      # 512
_NS = 8208                        # input rows per core
_MS = 512                        # output rows per core
_NCHUNK = 16                      # 1MB DMA chunks of 512 rows
_NQ = 4                           # psum tiles of 128 outputs
_NU = 17                          # weight matrices per psum tile

_cache = {}


def _dtype():
    import concourse.mybir as mybir

    name = os.environ.get("BASS_X_DTYPE", "float32r")
    return {"float32": mybir.dt.float32, "float32r": mybir.dt.float32r}[name]


def _build():
    if "nc" in _cache:
        return _cache["nc"]

    import concourse.bass as bass
    import concourse.mybir as mybir
    import concourse.tile as tile
    from concourse import bacc

    DT = _dtype()
    f32 = mybir.dt.float32

    nc = bacc.Bacc(None, target_bir_lowering=False, debug=False)
    xs = nc.dram_tensor("xs", [_NS, _F], DT, kind="ExternalInput")
    wbufd = nc.dram_tensor("wbufd", [128, _NU * 128], DT, kind="ExternalInput")
    biasd = nc.dram_tensor("biasd", [1, _F], f32, kind="ExternalInput")
    y = nc.dram_tensor("y", [_MS, _F], f32, kind="ExternalOutput")

    with tile.TileContext(nc) as tc:
        with (
            tc.tile_pool(name="xp", bufs=1) as xp,
            tc.tile_pool(name="wp", bufs=1) as wp,
            tc.tile_pool(name="pp", bufs=4, space=bass.MemorySpace.PSUM) as pp,
            tc.tile_pool(name="op", bufs=4) as op,
        ):
            # Weights + bias first so they are on-chip before compute starts.
            wbuf = wp.tile([128, _NU * 128], DT, tag="wbuf")
            nc.sync.dma_start(wbuf[:], wbufd.ap())
            bias_sb = wp.tile([1, _F], f32, tag="bias")
            nc.sync.dma_start(bias_sb[:], biasd.ap())
            ones = wp.tile([1, 128], f32, tag="ones")
            nc.gpsimd.memset(ones[:], 1.0)

            # Input x: 16 chunks of 512 rows as [128 part, 4 x-tiles, 512] + 16-row tail.
            xcs = []
            for c in range(_NCHUNK):
                t = xp.tile([128, 4, _F], DT, tag=f"x{c}")
                nc.sync.dma_start(
                    t[:],
                    xs.ap()[512 * c : 512 * (c + 1), :].rearrange(
                        "(j p) f -> p j f", p=128
                    ),
                )
                xcs.append(t)
            xtail = xp.tile([16, _F], DT, tag="xtail")
            nc.sync.dma_start(xtail[:], xs.ap()[8192:8208, :])

            # Compute: per psum tile q, 16 main + 1 window-tail + 1 bias matmul.
            for q in range(_NQ):
                ps = pp.tile([128, _F], f32)
                for u in range(16):
                    ti = 16 * q + u
                    rhs = xcs[ti // 4][:, ti % 4, :]
                    nc.tensor.matmul(
                        ps[:],
                        wbuf[:, 128 * u : 128 * (u + 1)],
                        rhs,
                        start=(u == 0),
                        stop=False,
                    )
                ti = 16 * (q + 1)
                rhs16 = xtail[:] if ti == 64 else xcs[ti // 4][0:16, ti % 4, :]
                nc.tensor.matmul(
                    ps[:], wbuf[0:16, 2048:2176], rhs16, start=False, stop=False
                )
                nc.tensor.matmul(ps[:], ones[:], bias_sb[:], start=False, stop=True)

                ot = op.tile([128, _F], f32)
                nc.vector.tensor_copy(ot[:], ps[:])
                nc.sync.dma_start(y.ap()[128 * q : 128 * (q + 1), :], ot[:])

    nc.compile()
    _cache["nc"] = nc
    return nc


def _host_prep(weight, pe):
    """Build the padded banded weight matrices [128, 17*128] and pe bias [1, 512]."""
    w = np.asarray(weight, dtype=np.float32)
    pe = np.asarray(pe, dtype=np.float32)
    p = np.arange(128)[:, None]
    j = np.arange(128)[None, :]
    wfull = np.zeros((128, _NU * 128), dtype=np.float32)
    for u in range(_NU):
        idx = 128 * u + p - 16 * j
        m = (idx >= 0) & (idx < _K)
        blk = np.zeros((128, 128), dtype=np.float32)
        blk[m] = w[idx[m]]
        wfull[:, 128 * u : 128 * (u + 1)] = blk
    bias = (w @ pe).astype(np.float32)          # [128]
    bias_row = np.tile(bias, _H)[None, :]       # [1, 512]
    return wfull, bias_row


LAST_RESULTS = None


def kernel(x, weight, pe, stride):
    global LAST_RESULTS
    from concourse.bass_utils import run_bass_kernel_spmd

    x = np.asarray(x, dtype=np.float32)
    assert x.shape == (_B, _N, _H, _D), x.shape
    assert int(stride) == _S

    nc = _build()
    wfull, bias_row = _host_prep(weight, pe)

    x2 = x.reshape(_B, _N, _F)
    in_maps = []
    for b in range(_B):
        for base in (0, _N - _NS):
            in_maps.append(
                {
                    "xs": np.ascontiguousarray(x2[b, base : base + _NS]),
                    "wbufd": wfull,
                    "biasd": bias_row,
                }
            )

    trace_cores = None
    if os.environ.get("BASS_TRACE"):
        tc_env = os.environ.get("BASS_TRACE_CORES", "0")
        trace_cores = [int(c) for c in tc_env.split(",")]
    res = run_bass_kernel_spmd(
        nc, in_maps, core_ids=list(range(8)), trace_cores=trace_cores
    )
    LAST_RESULTS = res

    out = np.empty((_B, _M, _H, _D), dtype=np.float32)
    for b in range(_B):
        y0 = res.results[2 * b]["y"].reshape(_MS, _H, _D)
        y1 = res.results[2 * b + 1]["y"].reshape(_MS, _H, _D)
        out[b, :_MS] = y0
        out[b, _MS:] = y1[1:]
    return out


# revision 4
# speedup vs baseline: 1.2194x; 1.2194x over previous
"""NSA-style block compression (sparse_attention) Trainium2 kernel.

y[b, m, h, :] = sum_{r<32} w[r] * (x[b, 16*m + r, h, :] + pe[r, :]),  M = 1023

Decomposition used on device (per core):
  - Shard: 8 cores = 4 batches x 2 sequence-halves. Each core gets a
    contiguous [8208, 512] slice of x[b] (rows = seq positions, cols = H*D)
    and produces 512 output rows ([512, 512]); halves overlap by one output
    row which the host drops.
  - x is DMA'd as 8 chunks of 1024 rows in [128, 8, 512] layout with rows
    interleaved so partition p holds rows 8p..8p+7 (16KB contiguous per
    partition -> large DMA descriptors).
  - The strided conv is expressed as 16 banded [128,128] weight matrices per
    128-output PSUM tile (one per (chunk-half, slice)), plus a [16,128] one
    for the window tail rows and a rank-1 matmul adding the pe bias
    (sum_r w[r]*pe[r,:], which factors out of the gather).
"""

import os
import sys

sys.path.insert(0, "/opt/trn_rl_repo")

import numpy as np

_B, _N, _H, _D = 4, 16384, 4, 128
_K, _S = 32, 16
_M = (_N - _K) // _S + 1          # 1023
_F = _H * _D                      # 512
_NS = 8208                        # input rows per core
_MS = 512                         # output rows per core
_NCHUNK = 8                       # 2MB DMA chunks of 1024 rows
_NQ = 4                           # psum tiles of 128 outputs
_NU = 17                          # weight matrices (16 main + window tail)

_cache = {}


def _dtype():
    import concourse.mybir as mybir

    name = os.environ.get("BASS_X_DTYPE", "float32r")
    return {"float32": mybir.dt.float32, "float32r": mybir.dt.float32r}[name]


def _build():
    if "nc" in _cache:
        return _cache["nc"]

    import concourse.bass as bass
    import concourse.mybir as mybir
    import concourse.tile as tile
    from concourse import bacc

    DT = _dtype()
    f32 = mybir.dt.float32

    nc = bacc.Bacc(None, target_bir_lowering=False, debug=False)
    xs = nc.dram_tensor("xs", [_NS, _F], DT, kind="ExternalInput")
    wbufd = nc.dram_tensor("wbufd", [128, _NU * 128], DT, kind="ExternalInput")
    biasd = nc.dram_tensor("biasd", [1, _F], f32, kind="ExternalInput")
    y = nc.dram_tensor("y", [_MS, _F], f32, kind="ExternalOutput")

    with tile.TileContext(nc) as tc:
        with (
            tc.tile_pool(name="xp", bufs=1) as xp,
            tc.tile_pool(name="wp", bufs=1) as wp,
            tc.tile_pool(name="pp", bufs=4, space=bass.MemorySpace.PSUM) as pp,
            tc.tile_pool(name="op", bufs=4) as op,
        ):
            # Weights + bias first so they are on-chip before compute starts.
            wbuf = wp.tile([128, _NU * 128], DT, tag="wbuf")
            nc.sync.dma_start(wbuf[:], wbufd.ap())
            bias_sb = wp.tile([1, _F], f32, tag="bias")
            nc.scalar.dma_start(bias_sb[:], biasd.ap())
            ones = wp.tile([1, 128], f32, tag="ones")
            nc.gpsimd.memset(ones[:], 1.0)

            # Window-tail rows for each psum tile (16 rows past its 2048-row span).
            bnds = []
            for q in range(_NQ):
                t = xp.tile([16, _F], DT, tag=f"bnd{q}")
                nc.scalar.dma_start(
                    t[:], xs.ap()[2048 * (q + 1) : 2048 * (q + 1) + 16, :]
                )
                bnds.append(t)

            # Input x: 8 chunks of 1024 rows as [128, 8, 512], row = 8p + s.
            xcs = []
            for c in range(_NCHUNK):
                t = xp.tile([128, 8, _F], DT, tag=f"x{c}")
                eng = nc.sync if c % 2 == 0 else nc.scalar
                eng.dma_start(
                    t[:],
                    xs.ap()[1024 * c : 1024 * (c + 1), :].rearrange(
                        "(p s) f -> p s f", s=8
                    ),
                )
                xcs.append(t)

            # Compute: per psum tile q, 16 main + 1 window-tail + 1 bias matmul.
            for q in range(_NQ):
                ps = pp.tile([128, _F], f32)
                for bi in range(16):
                    cc, s = bi // 8, bi % 8
                    rhs = xcs[2 * q + cc][:, s, :]
                    nc.tensor.matmul(
                        ps[:],
                        wbuf[:, 128 * bi : 128 * (bi + 1)],
                        rhs,
                        start=(bi == 0),
                        stop=False,
                    )
                nc.tensor.matmul(
                    ps[:], wbuf[0:16, 2048:2176], bnds[q][:], start=False, stop=False
                )
                nc.tensor.matmul(ps[:], ones[:], bias_sb[:], start=False, stop=True)

                ot = op.tile([128, _F], f32)
                nc.vector.tensor_copy(ot[:], ps[:])
                nc.sync.dma_start(y.ap()[128 * q : 128 * (q + 1), :], ot[:])

    nc.compile()
    _cache["nc"] = nc
    return nc


def _host_prep(weight, pe):
    """Build the banded weight matrices [128, 17*128] and pe bias [1, 512]."""
    w = np.asarray(weight, dtype=np.float32)
    pe = np.asarray(pe, dtype=np.float32)
    p = np.arange(128)[:, None]
    j = np.arange(128)[None, :]
    wfull = np.zeros((128, _NU * 128), dtype=np.float32)
    for bi in range(16):
        cc, s = bi // 8, bi % 8
        idx = 1024 * cc + 8 * p + s - 16 * j
        m = (idx >= 0) & (idx < _K)
        blk = np.zeros((128, 128), dtype=np.float32)
        blk[m] = w[idx[m]]
        wfull[:, 128 * bi : 128 * (bi + 1)] = blk
    # Window tail: rows 2048..2063 feed output 127 with the second half of w.
    idx = 2048 + p - 16 * j
    m = (idx >= 0) & (idx < _K)
    blk = np.zeros((128, 128), dtype=np.float32)
    blk[m] = w[idx[m]]
    wfull[:, 2048:2176] = blk
    bias = (w @ pe).astype(np.float32)          # [128]
    bias_row = np.tile(bias, _H)[None, :]       # [1, 512]
    return wfull, bias_row


LAST_RESULTS = None


def kernel(x, weight, pe, stride):
    global LAST_RESULTS
    from concourse.bass_utils import run_bass_kernel_spmd

    x = np.asarray(x, dtype=np.float32)
    assert x.shape == (_B, _N, _H, _D), x.shape
    assert int(stride) == _S

    nc = _build()
    wfull, bias_row = _host_prep(weight, pe)

    x2 = x.reshape(_B, _N, _F)
    in_maps = []
    for b in range(_B):
        for base in (0, _N - _NS):
            in_maps.append(
                {
                    "xs": np.ascontiguousarray(x2[b, base : base + _NS]),
                    "wbufd": wfull,
                    "biasd": bias_row,
                }
            )

    trace_cores = None
    if os.environ.get("BASS_TRACE"):
        tc_env = os.environ.get("BASS_TRACE_CORES", "0")
        trace_cores = [int(c) for c in tc_env.split(",")]
    res = run_bass_kernel_spmd(
        nc, in_maps, core_ids=list(range(8)), trace_cores=trace_cores
    )
    LAST_RESULTS = res

    out = np.empty((_B, _M, _H, _D), dtype=np.float32)
    for b in range(_B):
        y0 = res.results[2 * b]["y"].reshape(_MS, _H, _D)
        y1 = res.results[2 * b + 1]["y"].reshape(_MS, _H, _D)
        out[b, :_MS] = y0
        out[b, _MS:] = y1[1:]
    return out


# revision 7
# speedup vs baseline: 1.2858x; 1.0545x over previous
"""NSA-style block compression (sparse_attention) Trainium2 kernel.

y[b, m, h, :] = sum_{r<32} w[r] * (x[b, 16*m + r, h, :] + pe[r, :]),  M = 1023

Decomposition used on device (per core):
  - Shard: 8 cores = 4 batches x 2 sequence-halves. Each core gets a
    contiguous [8208, 512] slice of x[b] (rows = seq positions, cols = H*D)
    and produces 512 output rows ([512, 512]); halves overlap by one output
    row which the host drops.
  - x is DMA'd as 8 chunks of 1024 rows in [128, 8, 512] layout with rows
    interleaved so partition p holds rows 8p..8p+7 (16KB contiguous per
    partition -> large DMA descriptors). The last chunk is fetched as 4
    sub-DMAs so the tail compute starts earlier.
  - The strided conv becomes, per 128-output PSUM tile, 16 matmuls with just
    8 distinct [128, 64] banded weights U_s[p, c] = w[8p + s - 16c] (the two
    chunk-halves write psum partitions 0:64 / 64:128 by translation
    symmetry), one [16, 64] matmul for the 16 window-tail rows, and the pe
    bias (sum_r w[r]*pe[r, :], factored out of the gather) added during the
    PSUM->SBUF evacuation against a DMA-broadcast bias tile.
"""

import os
import sys

sys.path.insert(0, "/opt/trn_rl_repo")

import numpy as np

_B, _N, _H, _D = 4, 16384, 4, 128
_K, _S = 32, 16
_M = (_N - _K) // _S + 1          # 1023
_F = _H * _D                      # 512
_NS = 8208                        # input rows per core
_MS = 512                         # output rows per core
_NCHUNK = 8                       # 2MB DMA chunks of 1024 rows
_NQ = 4                           # psum tiles of 128 outputs
_WCOLS = 8 * 192 + 128            # 8 padded main strips + window-tail block

_cache = {}


def _dtype():
    import concourse.mybir as mybir

    name = os.environ.get("BASS_X_DTYPE", "float32r")
    return {"float32": mybir.dt.float32, "float32r": mybir.dt.float32r}[name]


def _build():
    if "nc" in _cache:
        return _cache["nc"]

    import concourse.bass as bass
    import concourse.mybir as mybir
    import concourse.tile as tile
    from concourse import bacc

    DT = _dtype()
    f32 = mybir.dt.float32

    nc = bacc.Bacc(None, target_bir_lowering=False, debug=False)
    xs = nc.dram_tensor("xs", [_NS, _F], DT, kind="ExternalInput")
    wbufd = nc.dram_tensor("wbufd", [128, _WCOLS], DT, kind="ExternalInput")
    biasd = nc.dram_tensor("biasd", [1, _F], f32, kind="ExternalInput")
    y = nc.dram_tensor("y", [_MS, _F], f32, kind="ExternalOutput")

    with tile.TileContext(nc) as tc:
        with (
            tc.tile_pool(name="xp", bufs=1) as xp,
            tc.tile_pool(name="wp", bufs=1) as wp,
            tc.tile_pool(name="pp", bufs=4, space=bass.MemorySpace.PSUM) as pp,
            tc.tile_pool(name="op", bufs=4) as op,
        ):
            # Small tensors first on the scalar ring: weights, bias, tail rows.
            wbuf = wp.tile([128, _WCOLS], DT, tag="wbuf")
            nc.scalar.dma_start(wbuf[:], wbufd.ap())
            bias_bc = wp.tile([128, _F], f32, tag="bias")
            nc.scalar.dma_start(bias_bc[:], biasd.ap().to_broadcast((128, _F)))
            bnds = []
            for q in range(_NQ):
                t = xp.tile([16, _F], DT, tag=f"bnd{q}")
                nc.scalar.dma_start(
                    t[:], xs.ap()[2048 * (q + 1) : 2048 * (q + 1) + 16, :]
                )
                bnds.append(t)

            # Input x: 8 chunks of 1024 rows as [128, 8, 512], row = 8p + s.
            # Even chunks on the sync ring, odd on the scalar ring; the last
            # chunk is split into 4 slice-pair sub-DMAs.
            xcs = []
            for c in range(_NCHUNK):
                t = xp.tile([128, 8, _F], DT, tag=f"x{c}")
                eng = nc.sync if c % 2 == 0 else nc.scalar
                src = xs.ap()[1024 * c : 1024 * (c + 1), :].rearrange(
                    "(p s) f -> p s f", s=8
                )
                if c == _NCHUNK - 1:
                    for k in range(4):
                        eng.dma_start(
                            t[:, 2 * k : 2 * k + 2, :], src[:, 2 * k : 2 * k + 2, :]
                        )
                else:
                    eng.dma_start(t[:], src)
                xcs.append(t)

            # Compute: per psum tile q, 16 main + 1 window-tail matmul; bias is
            # added during evacuation.
            for q in range(_NQ):
                ps = pp.tile([128, _F], f32)
                for bi in range(16):
                    cc, s = bi // 8, bi % 8
                    rhs = xcs[2 * q + cc][:, s, :]
                    off = 192 * s + 64 * (1 - cc)
                    nc.tensor.matmul(
                        ps[:],
                        wbuf[:, off : off + 128],
                        rhs,
                        start=(bi == 0),
                        stop=False,
                    )
                nc.tensor.matmul(
                    ps[:], wbuf[0:16, 1536:1664], bnds[q][:],
                    start=False, stop=True,
                )

                ot = op.tile([128, _F], f32)
                nc.vector.tensor_add(ot[:], ps[:], bias_bc[:])
                nc.sync.dma_start(y.ap()[128 * q : 128 * (q + 1), :], ot[:])

    nc.compile()
    _cache["nc"] = nc
    return nc


def _host_prep(weight, pe):
    """Build the banded weight blocks [128, 8*64+64] and pe bias [1, 512]."""
    w = np.asarray(weight, dtype=np.float32)
    pe = np.asarray(pe, dtype=np.float32)
    p = np.arange(128)[:, None]
    c = np.arange(64)[None, :]
    wfull = np.zeros((128, _WCOLS), dtype=np.float32)
    for s in range(8):
        idx = 8 * p + s - 16 * c
        m = (idx >= 0) & (idx < _K)
        blk = np.zeros((128, 64), dtype=np.float32)
        blk[m] = w[idx[m]]
        # Strip layout [zeros(64) | U_s | zeros(64)]: slices [64:192] / [0:128]
        # give the band at block-cols 0..63 (half 0) or 64..127 (half 1).
        wfull[:, 192 * s + 64 : 192 * s + 128] = blk
        # Output 63's window crosses the chunk boundary: rows 1024..1039 (the
        # second chunk-half's rows 8p+s < 16) contribute w[16+8p+s] at the
        # half-1 slice's column 63, which lives in the strip's left pad.
        wfull[0, 192 * s + 63] = w[16 + s]
        wfull[1, 192 * s + 63] = w[24 + s]
    # Window tail: row 2048+p feeds output column 127 (m' = 128q+127) with the
    # second half of w.
    wfull[:16, 1536 + 127] = w[16:32]
    bias = (w @ pe).astype(np.float32)          # [128]
    bias_row = np.tile(bias, _H)[None, :]       # [1, 512]
    return wfull, bias_row


LAST_RESULTS = None


def kernel(x, weight, pe, stride):
    global LAST_RESULTS
    from concourse.bass_utils import run_bass_kernel_spmd

    x = np.asarray(x, dtype=np.float32)
    assert x.shape == (_B, _N, _H, _D), x.shape
    assert int(stride) == _S

    nc = _build()
    wfull, bias_row = _host_prep(weight, pe)

    x2 = x.reshape(_B, _N, _F)
    in_maps = []
    for b in range(_B):
        for base in (0, _N - _NS):
            in_maps.append(
                {
                    "xs": np.ascontiguousarray(x2[b, base : base + _NS]),
                    "wbufd": wfull,
                    "biasd": bias_row,
                }
            )

    trace_cores = None
    if os.environ.get("BASS_TRACE"):
        tc_env = os.environ.get("BASS_TRACE_CORES", "0")
        trace_cores = [int(c) for c in tc_env.split(",")]
    res = run_bass_kernel_spmd(
        nc, in_maps, core_ids=list(range(8)), trace_cores=trace_cores
    )
    LAST_RESULTS = res

    out = np.empty((_B, _M, _H, _D), dtype=np.float32)
    for b in range(_B):
        y0 = res.results[2 * b]["y"].reshape(_MS, _H, _D)
        y1 = res.results[2 * b + 1]["y"].reshape(_MS, _H, _D)
        out[b, :_MS] = y0
        out[b, _MS:] = y1[1:]
    return out


# revision 8
# speedup vs baseline: 1.3541x; 1.0531x over previous
"""NSA-style block compression (sparse_attention) Trainium2 kernel.

y[b, m, h, :] = sum_{r<32} w[r] * (x[b, 16*m + r, h, :] + pe[r, :]),  M = 1023

Decomposition used on device (per core):
  - Shard: 8 cores = 4 batches x 2 sequence-halves. Each core gets a
    contiguous [8208, 512] slice of x[b] (rows = seq positions, cols = H*D)
    and produces 512 output rows ([512, 512]); halves overlap by one output
    row which the host drops.
  - x is DMA'd as 8 chunks of 1024 rows in [128, 8, 512] layout with rows
    interleaved so partition p holds rows 8p..8p+7 (16KB contiguous per
    partition -> large DMA descriptors). The last chunk is fetched as 4
    sub-DMAs so the tail compute starts earlier.
  - Each chunk feeds one 64-output PSUM tile: 8 matmuls with the banded
    weights U_s[p, c] = w[8p + s - 16c] (shared across tiles by translation
    symmetry) plus one [16, 64] matmul for the 16 window-tail rows (gathered
    host-side into a small side tensor). The pe bias (sum_r w[r]*pe[r, :],
    which factors out of the gather) is added during PSUM->SBUF evacuation
    against a DMA-broadcast bias tile.
"""

import os
import sys

sys.path.insert(0, "/opt/trn_rl_repo")

import numpy as np

_B, _N, _H, _D = 4, 16384, 4, 128
_K, _S = 32, 16
_M = (_N - _K) // _S + 1          # 1023
_F = _H * _D                      # 512
_NS = 8208                        # input rows per core
_MS = 512                         # output rows per core
_NCHUNK = 8                       # 2MB DMA chunks of 1024 rows
_WCOLS = 8 * 64 + 64              # 8 U_s blocks + window-tail block

_cache = {}


def _dtype():
    import concourse.mybir as mybir

    name = os.environ.get("BASS_X_DTYPE", "float32r")
    return {"float32": mybir.dt.float32, "float32r": mybir.dt.float32r}[name]


def _build():
    if "nc" in _cache:
        return _cache["nc"]

    import concourse.bass as bass
    import concourse.mybir as mybir
    import concourse.tile as tile
    from concourse import bacc

    DT = _dtype()
    f32 = mybir.dt.float32

    nc = bacc.Bacc(None, target_bir_lowering=False, debug=False)
    xs = nc.dram_tensor("xs", [_NS, _F], DT, kind="ExternalInput")
    wbufd = nc.dram_tensor("wbufd", [128, _WCOLS], DT, kind="ExternalInput")
    biasd = nc.dram_tensor("biasd", [1, _F], f32, kind="ExternalInput")
    bndd = nc.dram_tensor("bndd", [16, _NCHUNK, _F], DT, kind="ExternalInput")
    y = nc.dram_tensor("y", [_MS, _F], f32, kind="ExternalOutput")

    with tile.TileContext(nc) as tc:
        with (
            tc.tile_pool(name="xp", bufs=1) as xp,
            tc.tile_pool(name="wp", bufs=1) as wp,
            tc.tile_pool(name="pp", bufs=8, space=bass.MemorySpace.PSUM) as pp,
            tc.tile_pool(name="op", bufs=8) as op,
        ):
            # Small tensors ride the SWDGE (gpsimd) queue, leaving both HWDGE
            # rings free for the x chunk stream.
            wbuf = wp.tile([128, _WCOLS], DT, tag="wbuf")
            nc.gpsimd.dma_start(wbuf[:], wbufd.ap())
            bias_bc = wp.tile([64, _F], f32, tag="bias")
            nc.gpsimd.dma_start(bias_bc[:], biasd.ap().to_broadcast((64, _F)))
            bndt = wp.tile([16, _NCHUNK, _F], DT, tag="bnd")
            nc.gpsimd.dma_start(bndt[:], bndd.ap())

            # Input x: 8 chunks of 1024 rows as [128, 8, 512], row = 8p + s.
            # Even chunks on the sync ring, odd on the scalar ring; the last
            # chunk is split into 4 slice-pair sub-DMAs.
            xcs = []
            for c in range(_NCHUNK):
                t = xp.tile([128, 8, _F], DT, tag=f"x{c}")
                eng = nc.sync if c % 2 == 0 else nc.scalar
                src = xs.ap()[1024 * c : 1024 * (c + 1), :].rearrange(
                    "(p s) f -> p s f", s=8
                )
                if c == _NCHUNK - 1:
                    for k in range(4):
                        eng.dma_start(
                            t[:, 2 * k : 2 * k + 2, :], src[:, 2 * k : 2 * k + 2, :]
                        )
                else:
                    eng.dma_start(t[:], src)
                xcs.append(t)

            # Compute: one 64-output psum tile per chunk: 8 main matmuls + 1
            # window-tail matmul; bias is added during evacuation.
            for c in range(_NCHUNK):
                ps = pp.tile([64, _F], f32)
                for s in range(8):
                    nc.tensor.matmul(
                        ps[:],
                        wbuf[:, 64 * s : 64 * (s + 1)],
                        xcs[c][:, s, :],
                        start=(s == 0),
                        stop=False,
                    )
                nc.tensor.matmul(
                    ps[:], wbuf[0:16, 512:576], bndt[:, c, :],
                    start=False, stop=True,
                )

                ot = op.tile([64, _F], f32)
                nc.vector.tensor_add(ot[:], ps[:], bias_bc[:])
                nc.sync.dma_start(y.ap()[64 * c : 64 * (c + 1), :], ot[:])

    nc.compile()
    _cache["nc"] = nc
    return nc


def _host_prep(weight, pe):
    """Build the banded weight blocks [128, 8*64+64] and pe bias [1, 512]."""
    w = np.asarray(weight, dtype=np.float32)
    pe = np.asarray(pe, dtype=np.float32)
    p = np.arange(128)[:, None]
    c = np.arange(64)[None, :]
    wfull = np.zeros((128, _WCOLS), dtype=np.float32)
    for s in range(8):
        idx = 8 * p + s - 16 * c
        m = (idx >= 0) & (idx < _K)
        blk = np.zeros((128, 64), dtype=np.float32)
        blk[m] = w[idx[m]]
        wfull[:, 64 * s : 64 * (s + 1)] = blk
    # Window tail: rows 1024(c+1)+p (p<16) feed output column 63 with the
    # second half of w.
    wfull[:16, 512 + 63] = w[16:32]
    bias = (w @ pe).astype(np.float32)          # [128]
    bias_row = np.tile(bias, _H)[None, :]       # [1, 512]
    return wfull, bias_row


LAST_RESULTS = None


def kernel(x, weight, pe, stride):
    global LAST_RESULTS
    from concourse.bass_utils import run_bass_kernel_spmd

    x = np.asarray(x, dtype=np.float32)
    assert x.shape == (_B, _N, _H, _D), x.shape
    assert int(stride) == _S

    nc = _build()
    wfull, bias_row = _host_prep(weight, pe)

    x2 = x.reshape(_B, _N, _F)
    in_maps = []
    for b in range(_B):
        for base in (0, _N - _NS):
            shard = np.ascontiguousarray(x2[b, base : base + _NS])
            # Window-tail rows per chunk, gathered host-side: [16, 8, 512].
            bnd = np.ascontiguousarray(
                shard.reshape(_NS // 16, 16, _F)[64::64][: _NCHUNK].transpose(1, 0, 2)
            )
            in_maps.append(
                {"xs": shard, "wbufd": wfull, "biasd": bias_row, "bndd": bnd}
            )

    trace_cores = None
    if os.environ.get("BASS_TRACE"):
        tc_env = os.environ.get("BASS_TRACE_CORES", "0")
        trace_cores = [int(c) for c in tc_env.split(",")]
    res = run_bass_kernel_spmd(
        nc, in_maps, core_ids=list(range(8)), trace_cores=trace_cores
    )
    LAST_RESULTS = res

    out = np.empty((_B, _M, _H, _D), dtype=np.float32)
    for b in range(_B):
        y0 = res.results[2 * b]["y"].reshape(_MS, _H, _D)
        y1 = res.results[2 * b + 1]["y"].reshape(_MS, _H, _D)
        out[b, :_MS] = y0
        out[b, _MS:] = y1[1:]
    return out
